# revision 1
# baseline (speedup 1.0000x reference)
"""ArcLengthLoss distributed Bass kernel for 8 TRN2 NeuronCores.

Reference computation (see problem spec):
    s = output[:, :, 0]                               # [32, 153]
    A = s[:, a1] - s[:, a2]; a_term = exp(A.mean(1))  # [32]
    b1 = s[:, direct]                                 # [32, NC]
    b2 = sum_l mask(l<seg_len) * s[:, pad_idx[:, l]]  # [32, NC]
    loss = (a_term + |b1-b2|.mean(1)).mean()

Strategy: the per-combo gather/sum is algebraically a matmul against a signed
count matrix:  B[b, c] = sum_k sT[k, b] * W[k, c]  with
W[k, c] = [direct[c] == k] - #{l < seg_len[c] : pad_idx[c, l] == k}.
W is built SPARSELY (18 writes per combo) with the GPSIMD local_scatter
instruction in [combo, section] layout, DMA-transposed to [section, combo],
and contracted on the TensorEngine with the tiny bf16 table.  abs+sum is a
fused VectorE/ScalarE reduction; final scalar assembly happens on the host
from per-core partial vectors (that is the unshard step).

Combos are sharded across the 8 cores (32768 per core after padding).
"""
import sys

if "/opt/trn_rl_repo" not in sys.path:
    sys.path.insert(0, "/opt/trn_rl_repo")

import numpy as np

import concourse.bass as bass  # noqa: F401  (bass types used via bacc/tile)
import concourse.bacc as bacc
import concourse.tile as tile
from concourse.tile import add_dep_helper
from concourse import mybir
from concourse.bass_utils import run_bass_kernel_spmd

# ---- problem constants (hardcoded per spec) ----
B = 32            # batch
S = 153           # sections
NA = 136          # a1/a2 pairs
NC = 261972       # combos
L = 17            # max segments per combo
CORES = 8
PERCORE = 32768   # padded combos per core
NTOT = PERCORE * CORES

# ---- kernel tiling ----
P = 128           # partitions (combos per group-column)
G = 8             # groups per scatter tile
TILE = P * G      # combos per tile = 1024
TILES = PERCORE // TILE  # 32
HIBASE = G * 128  # 1024: start of HI region inside a Wt row
WTW = G * 128 + G * 32   # 1280: Wt width (LO 8*128 | HI 8*32)
NI = 18 * G       # scatter indices per partition per tile

_DT = mybir.dt
_CACHE = {}


M = 8             # tiles per mega preprocessing op


def build_nc():
    """Build + compile the per-core Bass graph (same graph on all 8 cores)."""
    nc = bacc.Bacc("TRN2", target_bir_lowering=False, debug=False,
                   num_devices=CORES)

    s_d = nc.dram_tensor("output", [B, S, 1], _DT.float32, kind="ExternalInput")
    a1_d = nc.dram_tensor("a1", [NA], _DT.int32, kind="ExternalInput")
    a2_d = nc.dram_tensor("a2", [NA], _DT.int32, kind="ExternalInput")
    tbl_d = nc.dram_tensor("tbl", [PERCORE, L + 2], _DT.int32,
                           kind="ExternalInput")

    o_d = nc.dram_tensor("outv", [128, 2], _DT.float32, kind="ExternalOutput")

    TT = mybir.AluOpType

    with tile.TileContext(nc) as tc:
        with (
            tc.tile_pool(name="const", bufs=1) as cpool,
            tc.tile_pool(name="mid", bufs=3) as mpool,
            tc.tile_pool(name="wts", bufs=6) as wpool,
            tc.tile_pool(name="acc", bufs=3) as apool,
            tc.tile_pool(name="psum", bufs=6, space="PSUM") as ppool,
            tc.tile_pool(name="psumA", bufs=1, space="PSUM") as papool,
        ):
            # ---- ALL passthrough DMAs first (single xbar mode transition
            # into the transpose-only steady state); pad table chunked over
            # both HWDGE queues so the first tiles can start early
            comb_all = cpool.tile([P, TILES, G, L + 2], _DT.int32)
            comb_src = tbl_d.ap().rearrange("(p t g) l -> p t (g l)",
                                            t=TILES, p=P)
            nc.scalar.dma_start(comb_all[:, 0:M], comb_src[:, 0:M])
            nc.sync.dma_start(comb_all[:, M:TILES], comb_src[:, M:TILES])
            s_sb = cpool.tile([B, S], _DT.float32)
            nc.scalar.dma_start(s_sb[:], s_d.ap().rearrange("b s o -> b (s o)"))
            a12r = cpool.tile([1, 2 * NA], _DT.int32)
            nc.scalar.dma_start(a12r[:, 0:NA], a1_d.ap().unsqueeze(0))
            nc.scalar.dma_start(a12r[:, NA:], a2_d.ap().unsqueeze(0))


            # ---- constants (all gpsimd setup collected, then ordered:
            # standard-lib ops -> partition_broadcasts -> warmup scatter,
            # so the steady-state loop pays zero ucode library reloads)
            _gs = []
            iota_l = cpool.tile([P, L], _DT.int32)          # 0..16
            _gs.append(nc.gpsimd.iota(iota_l[:], pattern=[[1, L]], base=0,
                           channel_multiplier=0))
            glo136 = cpool.tile([P, G * L], _DT.int32)      # g*128 x17
            _gs.append(nc.gpsimd.iota(glo136[:], pattern=[[128, G], [0, L]], base=0,
                           channel_multiplier=0))
            ghi136 = cpool.tile([P, G * L], _DT.int32)      # HIBASE-128+32g x17
            _gs.append(nc.gpsimd.iota(ghi136[:], pattern=[[32, G], [0, L]],
                           base=HIBASE - 128, channel_multiplier=0))
            c128 = cpool.tile([P, 1], _DT.int32)
            _gs.append(nc.gpsimd.memset(c128[:], 128))
            dump136 = cpool.tile([P, G * L], _DT.int16)     # HIBASE+25+32g x17
            _gs.append(nc.gpsimd.iota(dump136[:], pattern=[[32, G], [0, L]],
                           base=HIBASE + 25, channel_multiplier=0))
            glo8 = cpool.tile([P, G], _DT.int32)            # g*128
            _gs.append(nc.gpsimd.iota(glo8[:], pattern=[[128, G]], base=0,
                           channel_multiplier=0))
            ghi8 = cpool.tile([P, G], _DT.int32)
            _gs.append(nc.gpsimd.iota(ghi8[:], pattern=[[32, G]], base=HIBASE - 128,
                           channel_multiplier=0))
            # scatter data: [pads 136 -> -1 | direct 8 -> +1]
            data_c = cpool.tile([P, 18 * G], _DT.bfloat16)
            _gs.append(nc.gpsimd.memset(data_c[:], -1.0))
            _gs.append(nc.gpsimd.memset(data_c[:, G * L:], 1.0))
            iota_c = cpool.tile([128, 1], _DT.float32)
            _gs.append(nc.gpsimd.iota(iota_c[:], pattern=[[0, 1]], base=0,
                           channel_multiplier=1,
                           allow_small_or_imprecise_dtypes=True))
            iota_ch = cpool.tile([32, 1], _DT.float32)
            _gs.append(nc.gpsimd.iota(iota_ch[:], pattern=[[0, 1]], base=128,
                           channel_multiplier=1,
                           allow_small_or_imprecise_dtypes=True))
            # warmup scatter last: loads the scatter ucode before the loop
            wdum = cpool.tile([16, 2], _DT.bfloat16)
            idum = cpool.tile([16, 2], _DT.int16)
            ddum = cpool.tile([16, 2], _DT.bfloat16)
            _gs.append(nc.gpsimd.iota(idum[:], pattern=[[1, 2]], base=0,
                                      channel_multiplier=0))
            _gs.append(nc.gpsimd.memset(ddum[:], 0.0))
            warm = nc.gpsimd.local_scatter(wdum[:], ddum[:], idum[:],
                                           channels=16, num_elems=2,
                                           num_idxs=2)
            for _i in _gs:
                add_dep_helper(warm.ins, _i.ins, sync=False,
                               reason="gpsimd lib grouping")
            # broadcast a1/a2 to all partitions via a K=1 matmul
            a12b16 = cpool.tile([1, 2 * NA], _DT.bfloat16)
            nc.vector.tensor_copy(a12b16[:], a12r[:])
            ones1 = cpool.tile([1, 128], _DT.bfloat16)
            nc.vector.memset(ones1[:], 1.0)

            # ---- table prep on the Scalar engine (DVE stays free for the
            # first mega-preprocess), then the only setup transposes
            s16 = cpool.tile([B, 256], _DT.bfloat16)
            nc.vector.memset(s16[:], 0.0)
            nc.vector.tensor_copy(s16[:, 0:S], s_sb[:])
            s16rep = cpool.tile([B, 128], _DT.bfloat16)
            for r in range(4):
                nc.vector.tensor_copy(s16rep[:, r * 32:(r + 1) * 32],
                                      s16[:, 128:160])
            sT_lo = cpool.tile([128, B], _DT.bfloat16)
            nc.sync.dma_start_transpose(sT_lo[:], s16[:, 0:128])
            sT_hi = cpool.tile([128, B], _DT.bfloat16)
            nc.sync.dma_start_transpose(sT_hi[:], s16rep[:])
            # masked hi-table variants (rows outside [32g,32g+32) zeroed)
            sT_hi_g = []
            for g in range(4):
                t_ = cpool.tile([128, B], _DT.bfloat16, tag=f"sT_hi_{g}")
                nc.vector.memset(t_[:], 0.0)
                nc.vector.tensor_copy(t_[g * 32:(g + 1) * 32, :],
                                      sT_hi[g * 32:(g + 1) * 32, :])
                sT_hi_g.append(t_)

            # ---- pipeline state
            bacc_t = cpool.tile([128, TILES // 2], _DT.float32)
            st = [dict() for _ in range(TILES)]
            ps_roll = {}
            meg = {}

            def sA_prep(q):
                idx16 = mpool.tile([P, M, 18 * G], _DT.int16, tag="idx16",
                                   name=f"idx16_{q}")
                m16 = mpool.tile([P, M, 18 * G], _DT.int16, tag="m16",
                                 name=f"m16_{q}")
                hfl = mpool.tile([P, M, 18 * G], _DT.int16, tag="hfl",
                                 name=f"hfl_{q}")
                hi16 = mpool.tile([P, M, 18 * G], _DT.int16, tag="hi16",
                                  name=f"hi16_{q}")
                if q == 0:
                    _prep_ops(0, idx16[:, 0:2], m16[:, 0:2], hfl[:, 0:2],
                              hi16[:, 0:2], 0, 2)
                    _prep_ops(0, idx16[:, 2:M], m16[:, 2:M], hfl[:, 2:M],
                              hi16[:, 2:M], 2, M - 2)
                else:
                    _prep_ops(q, idx16[:], m16[:], hfl[:], hi16[:], 0, M)
                meg[q] = idx16

            def _prep_ops(q, idx16, m16, hfl, hi16, m0, MS):
                t0 = q * M + m0
                GL = G * L
                pads = comb_all[:, t0:t0 + MS, :, 0:L]
                segs = comb_all[:, t0:t0 + MS, :, L]
                dirs = comb_all[:, t0:t0 + MS, :, L + 1]

                def v4(ap2):  # [P, x*GL] slice -> [P, MS, G, L] view
                    return ap2.rearrange("p m (g l) -> p m g l", g=G)

                m16v = v4(m16[:, :, 0:GL])
                hflv = v4(hfl[:, :, 0:GL])
                hi16v = v4(hi16[:, :, 0:GL])
                idxv = v4(idx16[:, :, 0:GL])
                nc.vector.tensor_tensor(
                    m16v,
                    iota_l[:].unsqueeze(1).unsqueeze(1).broadcast_to(
                        [P, MS, G, L]),
                    segs.unsqueeze(3).broadcast_to([P, MS, G, L]),
                    op=TT.is_ge)
                nc.vector.tensor_tensor(
                    hflv, pads,
                    c128[:].unsqueeze(1).unsqueeze(1).broadcast_to(
                        [P, MS, G, L]),
                    op=TT.is_ge)
                nc.vector.tensor_tensor(
                    idxv, pads,
                    glo136[:].rearrange("p (g l) -> p g l", g=G).unsqueeze(
                        1).broadcast_to([P, MS, G, L]),
                    op=TT.add)
                nc.vector.tensor_tensor(
                    hi16v, pads,
                    ghi136[:].rearrange("p (g l) -> p g l", g=G).unsqueeze(
                        1).broadcast_to([P, MS, G, L]),
                    op=TT.add)
                nc.vector.copy_predicated(idxv, hflv, hi16v)
                nc.vector.copy_predicated(
                    idxv, m16v,
                    dump136[:].rearrange("p (g l) -> p g l", g=G).unsqueeze(
                        1).broadcast_to([P, MS, G, L]))
                # direct (3D [P, MS, G])
                nc.vector.tensor_tensor(
                    hfl[:, :, GL:GL + G], dirs,
                    c128[:].unsqueeze(1).broadcast_to([P, MS, G]),
                    op=TT.is_ge)
                nc.vector.tensor_tensor(
                    hi16[:, :, GL:GL + G], dirs,
                    ghi8[:].unsqueeze(1).broadcast_to([P, MS, G]),
                    op=TT.add)
                nc.vector.tensor_tensor(
                    idx16[:, :, GL:GL + G], dirs,
                    glo8[:].unsqueeze(1).broadcast_to([P, MS, G]),
                    op=TT.add)
                nc.vector.copy_predicated(idx16[:, :, GL:GL + G],
                                          hfl[:, :, GL:GL + G],
                                          hi16[:, :, GL:GL + G])

            def s2_scatter(t):
                d = st[t]
                wt = wpool.tile([P, WTW], _DT.bfloat16, tag="wt",
                                name=f"wt_{t}")
                nc.gpsimd.local_scatter(
                    wt[:], data_c[:], meg[t // M][:, t % M, :],
                    channels=P, num_elems=WTW, num_idxs=NI)
                d["wt"] = wt

            def s3_transpose(t):
                d = st[t]
                # single batched block transpose:
                # out[s, b, p] = wt[p, b*128 + s]  for all 10 col-blocks
                wT = wpool.tile([128, WTW], _DT.bfloat16, tag="wT",
                                name=f"wT_{t}")
                nc.sync.dma_start_transpose(
                    wT[:].rearrange("s (b p) -> s b p", b=WTW // 128),
                    d["wt"][:, :])
                d["wT"] = wT

            def s4_matmul(t):
                d = st[t]
                if t % 2 == 0:
                    ps_roll["ps"] = ppool.tile([128, 512], _DT.float32,
                                               tag="psB4", name=f"psB4_{t}")
                psum = ps_roll["ps"]
                d["psum"] = psum
                wT = d["wT"]
                for j in range(2):
                    a = (2 * t + j) % 4
                    sub = psum[32 * a:32 * (a + 1), :]
                    nc.tensor.matmul(
                        sub, sT_lo[:], wT[:, j * 512:(j + 1) * 512],
                        start=True, stop=False, skip_group_check=True,
                        tile_position=(0, 32 * a))
                    for gg in range(4):
                        nc.tensor.matmul(
                            sub[:, gg * 128:(gg + 1) * 128], sT_hi_g[gg][:],
                            wT[:, HIBASE + j * 128:HIBASE + (j + 1) * 128],
                            start=False, stop=True, skip_group_check=True,
                            tile_position=(0, 32 * a))

            def s5_reduce(t):
                if t % 2 == 1:
                    psum = st[t]["psum"]
                    col = t // 2
                    trash = apool.tile([128, 512], _DT.bfloat16,
                                       tag="trash", name=f"trash_{t}")
                    nc.scalar.activation(
                        trash[:], psum[:],
                        mybir.ActivationFunctionType.Abs,
                        accum_out=bacc_t[:, col:col + 1])
                st[t] = None

            # software-pipelined emission, deepest stage first; the mega
            # preprocess for group q is emitted M tiles ahead
            def s1_prep(t):
                if t % M == 0:
                    sA_prep(t // M)

            asum = cpool.tile([B, 1], _DT.float32)

            def emit_a_term():
                psbc = papool.tile([128, 2 * NA], _DT.float32, tag="psbc")
                nc.tensor.matmul(psbc[:], ones1[:], a12b16[:],
                                 start=True, stop=True)
                wa_lo = cpool.tile([128, NA], _DT.bfloat16)
                oh2 = cpool.tile([128, NA], _DT.bfloat16)
                nc.vector.tensor_scalar(wa_lo[:], psbc[:, 0:NA], iota_c[:],
                                        None, op0=TT.is_equal)
                nc.vector.tensor_scalar(oh2[:], psbc[:, NA:], iota_c[:], None,
                                        op0=TT.is_equal)
                nc.vector.tensor_tensor(wa_lo[:], wa_lo[:], oh2[:],
                                        op=TT.subtract)
                wa_hi = cpool.tile([32, NA], _DT.bfloat16)
                oh2h = cpool.tile([32, NA], _DT.bfloat16)
                nc.vector.tensor_scalar(wa_hi[:], psbc[0:32, 0:NA], iota_ch[:],
                                        None, op0=TT.is_equal)
                nc.vector.tensor_scalar(oh2h[:], psbc[0:32, NA:], iota_ch[:],
                                        None, op0=TT.is_equal)
                nc.vector.tensor_tensor(wa_hi[:], wa_hi[:], oh2h[:],
                                        op=TT.subtract)
                psa = papool.tile([B, NA], _DT.float32, tag="psa")
                nc.tensor.matmul(psa[:], sT_lo[:], wa_lo[:],
                                 start=True, stop=False)
                nc.tensor.matmul(psa[:], sT_hi_g[0][0:32, :], wa_hi[:],
                                 start=False, stop=True)
                nc.vector.tensor_reduce(asum[:], psa[:],
                                        axis=mybir.AxisListType.X,
                                        op=mybir.AluOpType.add)

            stages = [s1_prep, s2_scatter, s3_transpose, s4_matmul, s5_reduce]
            NS = len(stages)
            for step in range(TILES + NS - 1):
                for si in reversed(range(NS)):
                    t = step - si
                    if 0 <= t < TILES:
                        stages[si](t)
                if step == 7:
                    emit_a_term()

            outv = cpool.tile([128, 2], _DT.float32)
            nc.vector.memset(outv[:], 0.0)
            nc.vector.tensor_reduce(outv[:, 0:1], bacc_t[:],
                                    axis=mybir.AxisListType.X,
                                    op=mybir.AluOpType.add)
            nc.vector.tensor_copy(outv[0:32, 1:2], asum[:])
            nc.scalar.dma_start(o_d.ap(), outv[:])

    nc.compile()
    return nc


def _host_fixup(direct, pad, seg):
    """Detect combos whose scatter targets collide (duplicate active pad
    sections, or direct == an active pad section).  Those cannot be expressed
    by the 0/±1 scatter; neutralize them on-device and return their row
    indices so the host computes their contribution exactly.  Zero rows for
    the reference tables (all active sections of a combo are distinct there).
    """
    n = direct.shape[0]
    lane = np.arange(L)[None, :]
    act = np.where(lane < seg[:, None], pad, 2000 + lane)  # distinct sentinels
    d_eff = np.where(direct < S, direct, 3000)
    t = np.concatenate([d_eff[:, None], act], axis=1)
    t.sort(axis=1)
    dup = (t[:, 1:] == t[:, :-1]).any(axis=1)
    return np.nonzero(dup)[0]


def prepare(inputs):
    """Shard + fix up inputs.  Returns (in_maps, host_abs)."""
    s = np.asarray(inputs["output"], dtype=np.float32)
    a1 = np.asarray(inputs["a1"], dtype=np.int32)
    a2 = np.asarray(inputs["a2"], dtype=np.int32)
    direct = np.asarray(inputs["direct"], dtype=np.int32).copy()
    pad = np.asarray(inputs["pad_idx"], dtype=np.int32).copy()
    seg = np.asarray(inputs["seg_len"], dtype=np.int32).copy()

    # general-correctness fallback for collision rows (none for the
    # reference tables)
    host_abs = 0.0
    bad = _host_fixup(direct, pad, seg)
    if bad.size:
        sv = s[:, :, 0]
        for c in bad:
            m = (np.arange(L) < seg[c]).astype(np.float32)
            b2 = (sv[:, pad[c]] * m[None, :]).sum(axis=1)
            host_abs += float(np.abs(sv[:, direct[c]] - b2).sum())
        direct[bad] = S       # -> dump slot, contributes 0 on device
        seg[bad] = 0

    # pad to NTOT with neutral rows
    npad = NTOT - direct.shape[0]
    direct_p = np.concatenate([direct, np.full(npad, S, np.int32)])
    pad_p = np.concatenate([pad, np.zeros((npad, L), np.int32)])
    seg_p = np.concatenate([seg, np.zeros(npad, np.int32)])

    comb = np.concatenate(
        [pad_p, seg_p[:, None], direct_p[:, None]], axis=1).astype(np.int32)
    in_maps = []
    for i in range(CORES):
        sl = slice(i * PERCORE, (i + 1) * PERCORE)
        in_maps.append({
            "output": s, "a1": a1, "a2": a2,
            "tbl": np.ascontiguousarray(comb[sl]),
        })
    return in_maps, host_abs


def combine(outs, host_abs):
    total_abs = host_abs + sum(float(outs[i]["outv"][:, 0].sum())
                               for i in range(CORES))
    mean_a = float(np.exp(outs[0]["outv"][0:B, 1] / NA).mean())
    val = mean_a + total_abs / (B * NC)
    return np.asarray(val, dtype=np.float32)


def get_nc():
    if "nc" not in _CACHE:
        _CACHE["nc"] = build_nc()
    return _CACHE["nc"]


def kernel(**inputs) -> np.ndarray:
    in_maps, host_abs = prepare(inputs)
    res = run_bass_kernel_spmd(get_nc(), in_maps, core_ids=list(range(CORES)))
    return combine(res.results, host_abs)



# revision 34
# speedup vs baseline: 1.2696x; 1.2696x over previous
"""ArcLengthLoss distributed Bass kernel for 8 TRN2 NeuronCores (v2).

Reference computation:
    s = output[:, :, 0]                               # [32, 153]
    A = s[:, a1] - s[:, a2]; a_term = exp(A.mean(1))  # [32]
    b1 = s[:, direct]                                 # [32, NC]
    b2 = sum_l mask(l<seg_len) * s[:, pad_idx[:, l]]  # [32, NC]
    loss = (a_term + |b1-b2|.mean(1)).mean()

The per-combo gather/sum is a matmul against a signed indicator matrix
W[sec, combo] built directly in [section-row, combo] layout (no on-device
transposes).  Sections are ranked by usage on the host:
  rows 0..31   most-used, sign-pure sections -> shipped as BITMASKS,
               expanded to bf16 on the Vector engine (4 cheap passes)
  rows 32..127 mid sections -> GPSIMD local_scatter from host-built CSR
  rows 128+    25 rare sections -> a K=32 second matmul on 2 dedicated
               "hi" tiles per core that hold every combo referencing them
W production is SPLIT between gpsimd (scatter, ~165 GB/s port) and plain
DMA of host-prebuilt dense bf16 tiles (HBM bw), running in parallel.
Per tile: one K=128 matmul pair (4-way PE tile_position packing), then
abs+accumulate drains alternating between Scalar ACT and Vector TTR.
Combos the scatter cannot express (duplicate targets / minority signs /
static-capacity overflow) are computed exactly on the host (14 rows for
the reference tables).
"""
import sys

if "/opt/trn_rl_repo" not in sys.path:
    sys.path.insert(0, "/opt/trn_rl_repo")

import numpy as np
import ml_dtypes

import concourse.bass as bass  # noqa: F401
import concourse.bacc as bacc
import concourse.tile as tile
from concourse.tile import add_dep_helper
from concourse import mybir
from concourse.bass_utils import run_bass_kernel_spmd

# ---- problem constants ----
B = 32
S = 153
L = 17
NA = 136
CORES = 8
TILE = 1024
T = 32                    # tiles per core
PERCORE = T * TILE
NTOT = PERCORE * CORES

NBM = 32                  # bitmask rows (partitions 96..127)
NSC = 96                  # scatter rows (partitions 0..95)
NI = 176                  # static num_idxs per scatter row
NIH = 64                  # static num_idxs per hi row
TD = 19                   # dense (DMA-streamed) tiles per core
TG = T - TD               # gpsimd tiles per core
N_HI_TILES = 2

def _set_mode(td):
    global TD, TG, _dense_flags, DENSE_TILES, GP_TILES, HI_TILES, HI_CAP
    global _GP_POS, _DN_POS, _HI_POS
    TD = td
    TG = T - TD
    _dense_flags = [(t * TD) // T != ((t + 1) * TD) // T for t in range(T)]
    DENSE_TILES = [t for t in range(T) if _dense_flags[t]]
    GP_TILES = [t for t in range(T) if not _dense_flags[t]]
    HI_TILES = GP_TILES[-N_HI_TILES:] if TG >= N_HI_TILES else []
    HI_CAP = len(HI_TILES) * TILE * CORES
    _GP_POS = {t: i for i, t in enumerate(GP_TILES)}
    _DN_POS = {t: i for i, t in enumerate(DENSE_TILES)}
    _HI_POS = {t: i for i, t in enumerate(HI_TILES)}


_set_mode(TD)

_DT = mybir.dt
_CACHE = {}
DRAIN_MIXED = False       # Vector TTR drain crashes HW (keep Scalar ACT)
WITH_SHAMT = True         # emit the stride-0-outer shamt iota
WITH_SIGDMA = True        # emit the partition-offset-96 SIG dma


def build_nc():
    nc = bacc.Bacc("TRN2", target_bir_lowering=False, debug=False,
                   num_devices=CORES)

    s_d = nc.dram_tensor("sTa", [160, B], _DT.float32, kind="ExternalInput")
    sig_d = nc.dram_tensor("SIG", [NBM, 1], _DT.float32, kind="ExternalInput")
    a12_d = nc.dram_tensor("a12", [1, 2 * NA], _DT.int32, kind="ExternalInput")
    wd_d = nc.dram_tensor("WD", [TD, 128, TILE], _DT.int16,
                          kind="ExternalInput")
    if TG:
        bits_d = nc.dram_tensor("BITS", [TG, NBM, TILE // 16], _DT.int16,
                                kind="ExternalInput")
        idx_d = nc.dram_tensor("IDX", [TG, NSC, NI], _DT.int16,
                               kind="ExternalInput")
        dat_d = nc.dram_tensor("DAT", [TG, NSC, NI], _DT.bfloat16,
                               kind="ExternalInput")
    if HI_TILES:
        hix_d = nc.dram_tensor("HIX", [N_HI_TILES, 32, NIH], _DT.int16,
                               kind="ExternalInput")
        hid_d = nc.dram_tensor("HID", [N_HI_TILES, 32, NIH], _DT.bfloat16,
                               kind="ExternalInput")
    o_d = nc.dram_tensor("outv", [128, 2], _DT.float32, kind="ExternalOutput")

    TT = mybir.AluOpType

    with tile.TileContext(nc) as tc:
        with (
            tc.tile_pool(name="const", bufs=1) as cpool,
            tc.tile_pool(name="wts", bufs=7) as wpool,
            tc.tile_pool(name="inp", bufs=5) as ipool,
            tc.tile_pool(name="psum", bufs=6, space="PSUM") as ppool,
            tc.tile_pool(name="psumA", bufs=1, space="PSUM") as papool,
        ):
            # ---- input DMAs (all plain passthrough; small ones first)
            sTl_f = cpool.tile([128, B], _DT.float32)
            nc.scalar.dma_start(sTl_f[:], s_d.ap()[0:128])
            sTh_f = cpool.tile([32, B], _DT.float32)
            nc.scalar.dma_start(sTh_f[:], s_d.ap()[128:160])
            sig_f = cpool.tile([128, 1], _DT.float32)
            if WITH_SIGDMA:
                nc.scalar.dma_start(sig_f[96:128, :], sig_d.ap())
            a12r = cpool.tile([1, 2 * NA], _DT.int32)
            nc.scalar.dma_start(a12r[:], a12_d.ap())
            if HI_TILES:
                hix_sb = cpool.tile([32, N_HI_TILES * NIH], _DT.int16)
                hid_sb = cpool.tile([32, N_HI_TILES * NIH], _DT.bfloat16)
                for i in range(N_HI_TILES):
                    nc.sync.dma_start(hix_sb[:, i * NIH:(i + 1) * NIH],
                                      hix_d.ap()[i])
                    nc.sync.dma_start(hid_sb[:, i * NIH:(i + 1) * NIH],
                                      hid_d.ap()[i])

            # ---- converts (vector)
            sT_lo = cpool.tile([128, B], _DT.bfloat16)
            nc.vector.tensor_copy(sT_lo[:], sTl_f[:])
            sT_hi = cpool.tile([128, B], _DT.bfloat16)
            nc.vector.memset(sT_hi[:], 0.0)
            nc.vector.tensor_copy(sT_hi[0:32, :], sTh_f[:])
            a12b16 = cpool.tile([1, 2 * NA], _DT.bfloat16)
            nc.vector.tensor_copy(a12b16[:], a12r[:])
            ones1 = cpool.tile([1, 128], _DT.bfloat16)
            nc.vector.memset(ones1[:], 1.0)

            # ---- gpsimd setup: standard-lib ops first, then every
            # local_scatter (library loads once before the steady loop)
            _gs = []
            iota_c = cpool.tile([128, 1], _DT.float32)
            _gs.append(nc.gpsimd.iota(iota_c[:], pattern=[[0, 1]], base=0,
                                      channel_multiplier=1,
                                      allow_small_or_imprecise_dtypes=True))
            iota_ch = cpool.tile([32, 1], _DT.float32)
            _gs.append(nc.gpsimd.iota(iota_ch[:], pattern=[[0, 1]], base=128,
                                      channel_multiplier=1,
                                      allow_small_or_imprecise_dtypes=True))
            shamt = cpool.tile([128, TILE], _DT.int16)
            if WITH_SHAMT:
                _gs.append(nc.gpsimd.iota(shamt[:],
                                          pattern=[[0, TILE // 16], [1, 16]],
                                          base=0, channel_multiplier=0))
            # warmup scatter loads the ucode library
            wdum = cpool.tile([16, 2], _DT.bfloat16)
            idum = cpool.tile([16, 2], _DT.int16)
            ddum = cpool.tile([16, 2], _DT.bfloat16)
            _gs.append(nc.gpsimd.iota(idum[:], pattern=[[1, 2]], base=0,
                                      channel_multiplier=0))
            _gs.append(nc.gpsimd.memset(ddum[:], 0.0))
            warm = nc.gpsimd.local_scatter(wdum[:], ddum[:], idum[:],
                                           channels=16, num_elems=2,
                                           num_idxs=2)
            for _i in _gs:
                add_dep_helper(warm.ins, _i.ins, sync=False,
                               reason="gpsimd lib grouping")
            # hi strips (built once; rows 32.. zeroed so K=128 matmuls are
            # uniform with the lo matmuls' accumulation groups)
            hi_w = []
            for i in range(len(HI_TILES)):
                hw = cpool.tile([128, TILE], _DT.bfloat16, tag=f"hi_w{i}")
                nc.vector.memset(hw[:], 0.0)
                nc.gpsimd.local_scatter(
                    hw[0:32, :], hid_sb[:, i * NIH:(i + 1) * NIH],
                    hix_sb[:, i * NIH:(i + 1) * NIH],
                    channels=32, num_elems=TILE, num_idxs=NIH)
                hi_w.append(hw)

            # ---- pipeline state
            bacc_t = cpool.tile([128, T // 2], _DT.float32)
            st = [dict() for _ in range(T)]
            ps_roll = {}
            dq = [nc.sync, nc.scalar]

            def s_load(t):
                d = st[t]
                if _dense_flags[t]:
                    w = wpool.tile([128, TILE], _DT.bfloat16, tag="w",
                                   name=f"w_{t}")
                    td = _DN_POS[t]
                    eng = dq[td % len(dq)]
                    eng.dma_start(w[:], wd_d.ap()[td].bitcast(_DT.bfloat16))
                    d["w"] = w
                else:
                    g = _GP_POS[t]
                    ix = ipool.tile([NSC, NI], _DT.int16, tag="ix",
                                    name=f"ix_{t}")
                    da = ipool.tile([NSC, NI], _DT.bfloat16, tag="da",
                                    name=f"da_{t}")
                    bt = ipool.tile([128, TILE // 16], _DT.int16, tag="bt",
                                    name=f"bt_{t}")
                    nc.scalar.dma_start(ix[:], idx_d.ap()[g])
                    nc.sync.dma_start(da[:], dat_d.ap()[g])
                    nc.scalar.dma_start(bt[96:128, :], bits_d.ap()[g])
                    d["ix"], d["da"], d["bt"] = ix, da, bt

            def s_build(t):
                if _dense_flags[t]:
                    return
                d = st[t]
                w = wpool.tile([128, TILE], _DT.bfloat16, tag="w",
                               name=f"w_{t}")
                nc.gpsimd.local_scatter(w[0:NSC, :], d["da"][:], d["ix"][:],
                                        channels=NSC, num_elems=TILE,
                                        num_idxs=NI)
                tmp = ipool.tile([128, TILE], _DT.int16, tag="tmp",
                                 name=f"tmp_{t}")
                tmp2 = ipool.tile([128, TILE], _DT.int16, tag="tmp2",
                                  name=f"tmp2_{t}")
                bt = d["bt"]
                nc.vector.tensor_tensor(
                    tmp[96:128, :].rearrange("p (w k) -> p w k", k=16),
                    bt[96:128, :].unsqueeze(2).broadcast_to(
                        [NBM, TILE // 16, 16]),
                    shamt[96:128, :].rearrange("p (w k) -> p w k", k=16),
                    op=TT.logical_shift_right)
                nc.vector.tensor_scalar(tmp2[96:128, :], tmp[96:128, :], 1,
                                        None, op0=TT.bitwise_and)
                nc.vector.tensor_copy(w[96:128, :], tmp2[96:128, :])
                nc.vector.tensor_scalar(w[96:128, :], w[96:128, :],
                                        sig_f[96:128, :], None, op0=TT.mult)
                d["w"] = w

            def s_mm(t):
                d = st[t]
                if t % 2 == 0:
                    ps_roll["ps"] = ppool.tile([128, 512], _DT.float32,
                                               tag="ps", name=f"ps_{t}")
                psum = ps_roll["ps"]
                d["psum"] = psum
                w = d["w"]
                hi = t in _HI_POS
                for j in range(2):
                    a = (2 * t + j) % 4
                    sub = psum[32 * a:32 * (a + 1), :]
                    nc.tensor.matmul(
                        sub, sT_lo[:], w[:, j * 512:(j + 1) * 512],
                        start=True, stop=not hi, skip_group_check=True,
                        tile_position=(0, 32 * a))
                    if hi:
                        hw = hi_w[_HI_POS[t]]
                        nc.tensor.matmul(
                            sub, sT_hi[:], hw[:, j * 512:(j + 1) * 512],
                            start=False, stop=True, skip_group_check=True,
                            tile_position=(0, 32 * a))

            def s_drain(t):
                if t % 2 == 1:
                    psum = st[t]["psum"]
                    col = t // 2
                    if (col % 2 == 0) or not DRAIN_MIXED:
                        trash = ipool.tile([128, 512], _DT.bfloat16,
                                           tag="trash", name=f"trash_{t}")
                        nc.scalar.activation(
                            trash[:], psum[:],
                            mybir.ActivationFunctionType.Abs,
                            accum_out=bacc_t[:, col:col + 1])
                    else:
                        ng = ipool.tile([128, 512], _DT.float32,
                                        tag="ng", name=f"ng_{t}")
                        trash = ipool.tile([128, 512], _DT.float32,
                                           tag="trashv", name=f"trashv_{t}")
                        nc.vector.tensor_scalar(ng[:], psum[:], -1.0, None,
                                                op0=TT.mult)
                        nc.vector.tensor_tensor_reduce(
                            trash[:], psum[:], ng[:], 1.0, 0.0,
                            op0=TT.max, op1=TT.add,
                            accum_out=bacc_t[:, col:col + 1])
                st[t] = None

            asum = cpool.tile([B, 1], _DT.float32)

            def emit_a_term():
                psbc = papool.tile([128, 2 * NA], _DT.float32, tag="psbc")
                nc.tensor.matmul(psbc[:], ones1[:], a12b16[:],
                                 start=True, stop=True)
                wa_lo = cpool.tile([128, NA], _DT.bfloat16)
                oh2 = cpool.tile([128, NA], _DT.bfloat16)
                nc.vector.tensor_scalar(wa_lo[:], psbc[:, 0:NA], iota_c[:],
                                        None, op0=TT.is_equal)
                nc.vector.tensor_scalar(oh2[:], psbc[:, NA:], iota_c[:], None,
                                        op0=TT.is_equal)
                nc.vector.tensor_tensor(wa_lo[:], wa_lo[:], oh2[:],
                                        op=TT.subtract)
                wa_hi = cpool.tile([32, NA], _DT.bfloat16)
                oh2h = cpool.tile([32, NA], _DT.bfloat16)
                nc.vector.tensor_scalar(wa_hi[:], psbc[0:32, 0:NA], iota_ch[:],
                                        None, op0=TT.is_equal)
                nc.vector.tensor_scalar(oh2h[:], psbc[0:32, NA:], iota_ch[:],
                                        None, op0=TT.is_equal)
                nc.vector.tensor_tensor(wa_hi[:], wa_hi[:], oh2h[:],
                                        op=TT.subtract)
                psa = papool.tile([B, NA], _DT.float32, tag="psa")
                nc.tensor.matmul(psa[:], sT_lo[:], wa_lo[:],
                                 start=True, stop=False)
                nc.tensor.matmul(psa[:], sT_hi[0:32, :], wa_hi[:],
                                 start=False, stop=True)
                nc.vector.tensor_reduce(asum[:], psa[:],
                                        axis=mybir.AxisListType.X,
                                        op=mybir.AluOpType.add)

            stages = [s_load, s_build, s_mm, s_drain]
            NS = len(stages)
            for step in range(T + NS - 1):
                for si in reversed(range(NS)):
                    t = step - si
                    if 0 <= t < T:
                        stages[si](t)
                if step == 6:
                    emit_a_term()

            outv = cpool.tile([128, 2], _DT.float32)
            nc.vector.memset(outv[:], 0.0)
            nc.vector.tensor_reduce(outv[:, 0:1], bacc_t[:],
                                    axis=mybir.AxisListType.X,
                                    op=mybir.AluOpType.add)
            nc.vector.tensor_copy(outv[0:B, 1:2], asum[:])
            nc.scalar.dma_start(o_d.ap(), outv[:])

    nc.compile()
    return nc


def prepare(inputs):
    """Host-side prep: rank sections, route combos, build device arrays."""
    s = np.asarray(inputs["output"], np.float32)[:, :, 0]
    a1 = np.asarray(inputs["a1"], np.int64)
    a2 = np.asarray(inputs["a2"], np.int64)
    direct = np.asarray(inputs["direct"], np.int64)
    pad = np.asarray(inputs["pad_idx"], np.int64)
    seg = np.asarray(inputs["seg_len"], np.int64)
    NCv = direct.shape[0]
    lane = np.arange(L)[None, :]
    act = lane < seg[:, None]

    padrefs = np.bincount(pad[act], minlength=S)
    dirrefs = np.bincount(direct, minlength=S)
    usage = padrefs + dirrefs
    order = np.argsort(-usage, kind="stable")
    rank = np.empty(S, np.int64)
    rank[order] = np.arange(S)
    # partition row of each rank: top-32 -> 96..127 (bitmask quadrant),
    # next 96 -> 0..95 (scatter), rest -> 128.. (hi strip)
    pr_of_rank = np.concatenate([96 + np.arange(NBM), np.arange(NSC),
                                 128 + np.arange(S - 128)])
    prow = pr_of_rank[rank]                     # section -> partition row

    sig_sec = np.where(padrefs >= dirrefs, -1.0, 1.0).astype(np.float32)
    bm_sec = np.zeros(S, bool)
    bm_sec[order[:NBM]] = True

    is_minor = bm_sec[direct] & (sig_sec[direct] < 0)
    padm = act & bm_sec[pad] & (sig_sec[pad] > 0)
    is_minor |= padm.any(1)

    a_ = np.where(act, pad, 2000 + lane)
    tcat = np.concatenate([np.where(direct < S, direct, 3000)[:, None], a_], 1)
    tcat.sort(axis=1)
    dup = (tcat[:, 1:] == tcat[:, :-1]).any(1)

    host = dup | is_minor

    hi_sec = np.zeros(S, bool)
    hi_sec[order[128:]] = True
    has_hi = hi_sec[direct] | (hi_sec[pad] & act).any(1)
    hi_mask = has_hi & ~host
    hi_idx = np.flatnonzero(hi_mask)
    if hi_idx.size > HI_CAP:
        host[hi_idx[HI_CAP:]] = True
        hi_idx = hi_idx[:HI_CAP]
    lo_idx = np.flatnonzero(~has_hi & ~host)

    rng = np.random.default_rng(9)
    core_of = np.full(NCv, -1, np.int64)
    tile_of = np.full(NCv, -1, np.int64)
    col_of = np.full(NCv, -1, np.int64)

    h = rng.permutation(hi_idx)
    hc = np.arange(h.size)
    core_of[h] = hc % CORES
    ht = hc // CORES
    tile_of[h] = np.asarray(HI_TILES)[ht % N_HI_TILES]
    col_of[h] = ht // N_HI_TILES

    p = rng.permutation(lo_idx)
    used = np.zeros((CORES, T), np.int64)
    if h.size:
        np.add.at(used, (core_of[h], tile_of[h]), 1)
    free_cols = TILE - used
    cc, tt = np.meshgrid(np.arange(CORES), np.arange(T), indexing="ij")
    reps = free_cols.ravel()
    slot_core = np.repeat(cc.ravel(), reps)
    slot_tile = np.repeat(tt.ravel(), reps)
    slot_col = (np.arange(reps.sum()) -
                np.repeat(np.cumsum(reps) - reps, reps) +
                np.repeat(used.ravel(), reps))
    assert p.size <= slot_core.size, "combo overflow"
    n = p.size
    core_of[p] = slot_core[:n]
    tile_of[p] = slot_tile[:n]
    col_of[p] = slot_col[:n]

    is_dense_tile = np.asarray(_dense_flags)

    def build_entries():
        dev = np.flatnonzero(core_of >= 0)
        pr, pl = np.nonzero(act[dev])
        e_combo = np.concatenate([dev[pr], dev])
        e_row = np.concatenate([prow[pad[dev[pr], pl]], prow[direct[dev]]])
        e_val = np.concatenate([np.full(pr.size, -1.0, np.float32),
                                np.full(dev.size, 1.0, np.float32)])
        return (e_combo, e_row, e_val, core_of[e_combo], tile_of[e_combo],
                col_of[e_combo])

    e_combo, e_row, e_val, e_core, e_tile, e_col = build_entries()

    # prune static-capacity overflow (whole combos -> host)
    def prune(sel, nrows, cap, rowbase, keymul):
        nonlocal e_combo, e_row, e_val, e_core, e_tile, e_col, host
        idxs = np.flatnonzero(sel)
        if idxs.size == 0:
            return False
        key = keymul[0](idxs)
        sort = np.argsort(key, kind="stable")
        ks = key[sort]
        if ks.size == 0:
            return False
        first = np.r_[0, np.flatnonzero(np.diff(ks)) + 1]
        counts = np.diff(np.r_[first, ks.size])
        pos = np.arange(ks.size) - np.repeat(first, counts)
        bad = sort[pos >= cap]
        if bad.size == 0:
            return False
        bad_combos = np.unique(e_combo[idxs[bad]])
        host[bad_combos] = True
        core_of[bad_combos] = -1
        keep = core_of[e_combo] >= 0
        e_combo, e_row, e_val = e_combo[keep], e_row[keep], e_val[keep]
        e_core, e_tile, e_col = e_core[keep], e_tile[keep], e_col[keep]
        return True

    gp_pos_arr = np.full(T, -1, np.int64)
    for i, t_ in enumerate(GP_TILES):
        gp_pos_arr[t_] = i
    dn_pos_arr = np.full(T, -1, np.int64)
    for i, t_ in enumerate(DENSE_TILES):
        dn_pos_arr[t_] = i
    hi_pos_arr = np.full(T, -1, np.int64)
    for i, t_ in enumerate(HI_TILES):
        hi_pos_arr[t_] = i

    for _ in range(3):
        sel = (e_row < NSC) & ~is_dense_tile[e_tile]
        km = [lambda ii: (e_core[ii] * TG + gp_pos_arr[e_tile[ii]]) * NSC
              + e_row[ii]]
        c1 = prune(sel, NSC, NI, NBM, km)
        sel = e_row >= 128
        km = [lambda ii: (e_core[ii] * N_HI_TILES + hi_pos_arr[e_tile[ii]])
              * 32 + (e_row[ii] - 128)]
        c2 = prune(sel, 32, NIH, 128, km)
        if not (c1 or c2):
            break

    dense_e = is_dense_tile[e_tile]
    WD = np.zeros((CORES, TD, 128, TILE), ml_dtypes.bfloat16)
    de = np.flatnonzero(dense_e)
    WD[e_core[de], dn_pos_arr[e_tile[de]], e_row[de], e_col[de]] = \
        e_val[de].astype(ml_dtypes.bfloat16)
    WD = WD.view(np.int16)

    bm_e = (~dense_e) & (e_row >= NSC) & (e_row < 128)
    BM = np.zeros((CORES, TG, NBM, TILE), bool)
    be = np.flatnonzero(bm_e)
    BM[e_core[be], gp_pos_arr[e_tile[be]], e_row[be] - NSC, e_col[be]] = True
    w16 = (1 << np.arange(16)).astype(np.int64)
    BITS = (BM.reshape(CORES, TG, NBM, TILE // 16, 16) @ w16).astype(
        np.uint16).view(np.int16)

    sc = (~dense_e) & (e_row < NSC)
    se = np.flatnonzero(sc)
    skey = (e_core[se] * TG + gp_pos_arr[e_tile[se]]) * NSC + e_row[se]
    sort = np.argsort(skey, kind="stable")
    ks = skey[sort]
    _, first_idx, counts = np.unique(ks, return_index=True, return_counts=True)
    pos = np.arange(ks.size) - np.repeat(first_idx, counts)
    IDX = np.full((CORES, TG, NSC, NI), -1, np.int16)
    DAT = np.zeros((CORES, TG, NSC, NI), ml_dtypes.bfloat16)
    ses = se[sort]
    IDX.reshape(-1, NI)[ks, pos] = e_col[ses].astype(np.int16)
    DAT.reshape(-1, NI)[ks, pos] = e_val[ses].astype(ml_dtypes.bfloat16)

    he = np.flatnonzero(e_row >= 128)
    hkey = ((e_core[he] * N_HI_TILES + hi_pos_arr[e_tile[he]]) * 32 +
            (e_row[he] - 128))
    sort = np.argsort(hkey, kind="stable")
    ks = hkey[sort]
    _, first_idx, counts = np.unique(ks, return_index=True, return_counts=True)
    pos = np.arange(ks.size) - np.repeat(first_idx, counts)
    HIX = np.full((CORES, N_HI_TILES, 32, NIH), -1, np.int16)
    HID = np.zeros((CORES, N_HI_TILES, 32, NIH), ml_dtypes.bfloat16)
    hes = he[sort]
    HIX.reshape(-1, NIH)[ks, pos] = e_col[hes].astype(np.int16)
    HID.reshape(-1, NIH)[ks, pos] = e_val[hes].astype(ml_dtypes.bfloat16)

    sTa = np.zeros((160, B), np.float32)
    sTa[prow] = s.T
    SIG = sig_sec[order[:NBM]].reshape(NBM, 1).astype(np.float32)
    a12 = np.concatenate([prow[a1], prow[a2]]).astype(np.int32).reshape(
        1, 2 * NA)

    hs = np.flatnonzero(host)
    host_abs = 0.0
    if hs.size:
        m = act[hs].astype(np.float32)
        b2 = np.einsum("bnl,nl->bn", s[:, pad[hs]], m)
        b1 = s[:, direct[hs]]
        host_abs = float(np.abs(b1 - b2).sum())

    in_maps = []
    for c in range(CORES):
        m = {"sTa": sTa, "SIG": SIG, "a12": a12, "WD": WD[c]}
        if TG:
            m.update({"BITS": BITS[c], "IDX": IDX[c], "DAT": DAT[c]})
        if HI_TILES:
            m.update({"HIX": HIX[c], "HID": HID[c]})
        in_maps.append(m)
    return in_maps, dict(NCv=NCv, host_abs=host_abs, n_host=int(hs.size))


def combine(outs, meta):
    total_abs = meta["host_abs"] + sum(float(outs[i]["outv"][:, 0].sum())
                                       for i in range(CORES))
    mean_a = float(np.exp(outs[0]["outv"][0:B, 1] / NA).mean())
    val = mean_a + total_abs / (B * meta["NCv"])
    return np.asarray(val, dtype=np.float32)


def get_nc():
    if "nc" not in _CACHE:
        _CACHE["nc"] = build_nc()
    return _CACHE["nc"]


def kernel(**inputs) -> np.ndarray:
    in_maps, meta = prepare(inputs)
    res = run_bass_kernel_spmd(get_nc(), in_maps, core_ids=list(range(CORES)))
    return combine(res.results, meta)


# revision 38
# speedup vs baseline: 2.2828x; 1.7981x over previous
"""ArcLengthLoss distributed Bass kernel for 8 TRN2 NeuronCores (v3).

Reference computation:
    s = output[:, :, 0]                               # [32, 153]
    A = s[:, a1] - s[:, a2]; a_term = exp(A.mean(1))  # [32]
    b1 = s[:, direct]                                 # [32, NC]
    b2 = sum_l mask(l<seg_len) * s[:, pad_idx[:, l]]  # [32, NC]
    loss = (a_term + |b1-b2|.mean(1)).mean()

The per-combo gather/sum is a matmul against a signed indicator matrix
W[sec, combo] built directly in [section-row, combo] layout.  Sections are
ranked by usage on the host:
  rows 0..95    mid-popularity sections -> GPSIMD local_scatter from a
                host-built CSR (idx|val int16, preloaded to SBUF once)
  rows 96..127  32 most-used sections (~70% of refs) -> dense bf16 strip
                DMA'd straight into the W tile (signs baked by host)
  rows 128+     25 rare sections -> K=32-worth second matmul on 2 "hi"
                tiles per core holding every combo that references them
W production is split between gpsimd (scatter port ~165 GB/s) and plain
HBM DMA: most tiles ship as host-prebuilt dense bf16, one mega-DMA per
consecutive dense run.  K=128 matmuls use 4-way PE tile_position packing;
abs+sum drains alternate Scalar ACT and a 3-pass Vector sequence.
Combos the scatter cannot express (duplicate targets, capacity overflow)
are computed exactly on the host (0 rows for the reference tables).
"""
import sys

if "/opt/trn_rl_repo" not in sys.path:
    sys.path.insert(0, "/opt/trn_rl_repo")

import numpy as np
import ml_dtypes

import concourse.bass as bass  # noqa: F401
import concourse.bacc as bacc
import concourse.tile as tile
from concourse.tile import add_dep_helper
from concourse import mybir
from concourse.bass_utils import run_bass_kernel_spmd

# ---- problem constants ----
B = 32
S = 153
L = 17
NA = 136
CORES = 8
TILE = 1024
T = 32                    # tiles per core
PERCORE = T * TILE
NTOT = PERCORE * CORES

NBM = 32                  # dense-strip rows (partitions 96..127)
NSC = 96                  # scatter rows (partitions 0..95)
NI = 176                  # static num_idxs per scatter row
NIH = 64                  # static num_idxs per hi row
TD = 27                   # dense (DMA-streamed) tiles per core
N_HI_TILES = 2
MAXRUN = 7                # max dense tiles per mega-DMA


def _set_mode(td):
    global TD, TG, _dense_flags, DENSE_TILES, GP_TILES, HI_TILES, HI_CAP
    global _GP_POS, _DN_POS, _HI_POS, DENSE_RUNS
    TD = td
    TG = T - TD
    _dense_flags = [(t * TD) // T != ((t + 1) * TD) // T for t in range(T)]
    DENSE_TILES = [t for t in range(T) if _dense_flags[t]]
    GP_TILES = [t for t in range(T) if not _dense_flags[t]]
    HI_TILES = GP_TILES[-N_HI_TILES:] if TG >= N_HI_TILES else []
    HI_CAP = len(HI_TILES) * TILE * CORES
    _GP_POS = {t: i for i, t in enumerate(GP_TILES)}
    _DN_POS = {t: i for i, t in enumerate(DENSE_TILES)}
    _HI_POS = {t: i for i, t in enumerate(HI_TILES)}
    # consecutive dense runs (t_start -> run length), split at MAXRUN
    DENSE_RUNS = {}
    run = []
    for t in range(T + 1):
        if t < T and _dense_flags[t]:
            run.append(t)
            if len(run) == MAXRUN:
                DENSE_RUNS[run[0]] = len(run)
                run = []
        else:
            if run:
                DENSE_RUNS[run[0]] = len(run)
            run = []


_set_mode(TD)

_DT = mybir.dt
_CACHE = {}
DRAIN_MIXED = True        # alternate Scalar ACT / 3-pass Vector drains


def build_nc():
    nc = bacc.Bacc("TRN2", target_bir_lowering=False, debug=False,
                   num_devices=CORES)

    s_d = nc.dram_tensor("sTa", [160, B], _DT.float32, kind="ExternalInput")
    a12_d = nc.dram_tensor("a12", [1, 2 * NA], _DT.int32, kind="ExternalInput")
    wd_d = nc.dram_tensor("WD", [TD, 128, TILE], _DT.int16,
                          kind="ExternalInput")
    if TG:
        bms_d = nc.dram_tensor("BMS", [TG, NBM, TILE], _DT.int16,
                               kind="ExternalInput")
        ixd_d = nc.dram_tensor("IXD", [TG, NSC, 2 * NI], _DT.int16,
                               kind="ExternalInput")
    if HI_TILES:
        hxd_d = nc.dram_tensor("HXD", [N_HI_TILES, 32, 2 * NIH], _DT.int16,
                               kind="ExternalInput")
    o_d = nc.dram_tensor("outv", [128, 2], _DT.float32, kind="ExternalOutput")

    TT = mybir.AluOpType

    with tile.TileContext(nc) as tc:
        with (
            tc.tile_pool(name="const", bufs=1) as cpool,
            tc.tile_pool(name="wts", bufs=3) as wpool,
            tc.tile_pool(name="gwts", bufs=4) as gpool,
            tc.tile_pool(name="drain", bufs=3) as dpool,
            tc.tile_pool(name="psum", bufs=6, space="PSUM") as ppool,
            tc.tile_pool(name="psumA", bufs=1, space="PSUM") as papool,
        ):
            # ---- input DMAs (small constants + full CSR preload)
            sTl_f = cpool.tile([128, B], _DT.float32)
            nc.scalar.dma_start(sTl_f[:], s_d.ap()[0:128])
            sTh_f = cpool.tile([32, B], _DT.float32)
            nc.scalar.dma_start(sTh_f[:], s_d.ap()[128:160])
            a12r = cpool.tile([1, 2 * NA], _DT.int32)
            nc.scalar.dma_start(a12r[:], a12_d.ap())
            if TG:
                ixd_all = cpool.tile([NSC, TG * 2 * NI], _DT.int16)
                nc.sync.dma_start(
                    ixd_all[:].rearrange("p (g c) -> p g c", g=TG),
                    ixd_d.ap().rearrange("g p c -> p g c"))
            if HI_TILES:
                hxd_sb = cpool.tile([32, N_HI_TILES * 2 * NIH], _DT.int16)
                nc.sync.dma_start(
                    hxd_sb[:].rearrange("p (g c) -> p g c", g=N_HI_TILES),
                    hxd_d.ap().rearrange("g p c -> p g c"))

            # ---- converts (vector)
            sT_lo = cpool.tile([128, B], _DT.bfloat16)
            nc.vector.tensor_copy(sT_lo[:], sTl_f[:])
            sT_hi = cpool.tile([128, B], _DT.bfloat16)
            nc.vector.memset(sT_hi[:], 0.0)
            nc.vector.tensor_copy(sT_hi[0:32, :], sTh_f[:])
            a12b16 = cpool.tile([1, 2 * NA], _DT.bfloat16)
            nc.vector.tensor_copy(a12b16[:], a12r[:])
            ones1 = cpool.tile([1, 128], _DT.bfloat16)
            nc.vector.memset(ones1[:], 1.0)

            # ---- gpsimd setup: std-lib ops first, then all local_scatters
            _gs = []
            iota_c = cpool.tile([128, 1], _DT.float32)
            _gs.append(nc.gpsimd.iota(iota_c[:], pattern=[[0, 1]], base=0,
                                      channel_multiplier=1,
                                      allow_small_or_imprecise_dtypes=True))
            iota_ch = cpool.tile([32, 1], _DT.float32)
            _gs.append(nc.gpsimd.iota(iota_ch[:], pattern=[[0, 1]], base=128,
                                      channel_multiplier=1,
                                      allow_small_or_imprecise_dtypes=True))
            wdum = cpool.tile([16, 2], _DT.bfloat16)
            idum = cpool.tile([16, 2], _DT.int16)
            ddum = cpool.tile([16, 2], _DT.bfloat16)
            _gs.append(nc.gpsimd.iota(idum[:], pattern=[[1, 2]], base=0,
                                      channel_multiplier=0))
            _gs.append(nc.gpsimd.memset(ddum[:], 0.0))
            warm = nc.gpsimd.local_scatter(wdum[:], ddum[:], idum[:],
                                           channels=16, num_elems=2,
                                           num_idxs=2)
            for _i in _gs:
                add_dep_helper(warm.ins, _i.ins, sync=False,
                               reason="gpsimd lib grouping")
            # hi strips, built once (rows 32.. zeroed: K=128 matmuls stay
            # uniform inside the accumulation groups)
            hi_w = []
            for i in range(len(HI_TILES)):
                hw = cpool.tile([128, TILE], _DT.bfloat16, tag=f"hi_w{i}")
                nc.vector.memset(hw[:], 0.0)
                base = i * 2 * NIH
                nc.gpsimd.local_scatter(
                    hw[0:32, :],
                    hxd_sb[:, base + NIH:base + 2 * NIH].bitcast(_DT.bfloat16),
                    hxd_sb[:, base:base + NIH],
                    channels=32, num_elems=TILE, num_idxs=NIH)
                hi_w.append(hw)

            # ---- pipeline state
            bacc_t = cpool.tile([128, T // 2], _DT.float32)
            st = [dict() for _ in range(T)]
            ps_roll = {}
            dq = [nc.sync, nc.scalar]

            def s_load(t):
                if _dense_flags[t]:
                    if t not in DENSE_RUNS:
                        return
                    k = DENSE_RUNS[t]
                    td = _DN_POS[t]
                    mega = wpool.tile([128, k * TILE], _DT.bfloat16,
                                      tag=f"mega{k}", name=f"mega_{t}")
                    eng = dq[td % 2]
                    eng.dma_start(
                        mega[:].rearrange("p (td c) -> p td c", td=k),
                        wd_d.ap()[td:td + k].rearrange(
                            "td p c -> p td c").bitcast(_DT.bfloat16))
                    for i in range(k):
                        st[t + i]["wt"] = mega
                        st[t + i]["off"] = i * TILE
                else:
                    g = _GP_POS[t]
                    w = gpool.tile([128, TILE], _DT.bfloat16, tag="w",
                                   name=f"w_{t}")
                    nc.scalar.dma_start(
                        w[96:128, :], bms_d.ap()[g].bitcast(_DT.bfloat16))
                    st[t]["wt"] = w
                    st[t]["off"] = 0

            def s_build(t):
                if _dense_flags[t]:
                    return
                d = st[t]
                g = _GP_POS[t]
                base = g * 2 * NI
                nc.gpsimd.local_scatter(
                    d["wt"][0:NSC, :],
                    ixd_all[:, base + NI:base + 2 * NI].bitcast(_DT.bfloat16),
                    ixd_all[:, base:base + NI],
                    channels=NSC, num_elems=TILE, num_idxs=NI)

            def s_mm(t):
                d = st[t]
                if t % 2 == 0:
                    ps_roll["ps"] = ppool.tile([128, 512], _DT.float32,
                                               tag="ps", name=f"ps_{t}")
                psum = ps_roll["ps"]
                d["psum"] = psum
                wt, off = d["wt"], d["off"]
                hi = t in _HI_POS
                for j in range(2):
                    a = (2 * t + j) % 4
                    sub = psum[32 * a:32 * (a + 1), :]
                    nc.tensor.matmul(
                        sub, sT_lo[:], wt[:, off + j * 512:off + (j + 1) * 512],
                        start=True, stop=not hi, skip_group_check=True,
                        tile_position=(0, 32 * a))
                    if hi:
                        hw = hi_w[_HI_POS[t]]
                        nc.tensor.matmul(
                            sub, sT_hi[:], hw[:, j * 512:(j + 1) * 512],
                            start=False, stop=True, skip_group_check=True,
                            tile_position=(0, 32 * a))

            def s_drain(t):
                if t % 2 == 1:
                    psum = st[t]["psum"]
                    col = t // 2
                    if (col % 2 == 0) or not DRAIN_MIXED:
                        trash = dpool.tile([128, 512], _DT.bfloat16,
                                           tag="trash", name=f"trash_{t}")
                        nc.scalar.activation(
                            trash[:], psum[:],
                            mybir.ActivationFunctionType.Abs,
                            accum_out=bacc_t[:, col:col + 1])
                    else:
                        ng = dpool.tile([128, 512], _DT.float32,
                                        tag="ng", name=f"ng_{t}")
                        ab = dpool.tile([128, 512], _DT.float32,
                                        tag="ab", name=f"ab_{t}")
                        nc.vector.tensor_scalar(ng[:], psum[:], -1.0, None,
                                                op0=TT.mult)
                        nc.vector.tensor_tensor(ab[:], psum[:], ng[:],
                                                op=TT.max)
                        nc.vector.tensor_reduce(bacc_t[:, col:col + 1], ab[:],
                                                axis=mybir.AxisListType.X,
                                                op=TT.add)
                st[t] = None

            asum = cpool.tile([B, 1], _DT.float32)

            def emit_a_term():
                psbc = papool.tile([128, 2 * NA], _DT.float32, tag="psbc")
                nc.tensor.matmul(psbc[:], ones1[:], a12b16[:],
                                 start=True, stop=True)
                wa_lo = cpool.tile([128, NA], _DT.bfloat16)
                oh2 = cpool.tile([128, NA], _DT.bfloat16)
                nc.vector.tensor_scalar(wa_lo[:], psbc[:, 0:NA], iota_c[:],
                                        None, op0=TT.is_equal)
                nc.vector.tensor_scalar(oh2[:], psbc[:, NA:], iota_c[:], None,
                                        op0=TT.is_equal)
                nc.vector.tensor_tensor(wa_lo[:], wa_lo[:], oh2[:],
                                        op=TT.subtract)
                wa_hi = cpool.tile([32, NA], _DT.bfloat16)
                oh2h = cpool.tile([32, NA], _DT.bfloat16)
                nc.vector.tensor_scalar(wa_hi[:], psbc[0:32, 0:NA], iota_ch[:],
                                        None, op0=TT.is_equal)
                nc.vector.tensor_scalar(oh2h[:], psbc[0:32, NA:], iota_ch[:],
                                        None, op0=TT.is_equal)
                nc.vector.tensor_tensor(wa_hi[:], wa_hi[:], oh2h[:],
                                        op=TT.subtract)
                psa = papool.tile([B, NA], _DT.float32, tag="psa")
                nc.tensor.matmul(psa[:], sT_lo[:], wa_lo[:],
                                 start=True, stop=False)
                nc.tensor.matmul(psa[:], sT_hi[0:32, :], wa_hi[:],
                                 start=False, stop=True)
                nc.vector.tensor_reduce(asum[:], psa[:],
                                        axis=mybir.AxisListType.X,
                                        op=mybir.AluOpType.add)

            stages = [s_load, s_build, s_mm, s_drain]
            NS = len(stages)
            for step in range(T + NS - 1):
                for si in reversed(range(NS)):
                    t = step - si
                    if 0 <= t < T:
                        stages[si](t)
                if step == 6:
                    emit_a_term()

            outv = cpool.tile([128, 2], _DT.float32)
            nc.vector.memset(outv[:], 0.0)
            nc.vector.tensor_reduce(outv[:, 0:1], bacc_t[:],
                                    axis=mybir.AxisListType.X,
                                    op=mybir.AluOpType.add)
            nc.vector.tensor_copy(outv[0:B, 1:2], asum[:])
            nc.scalar.dma_start(o_d.ap(), outv[:])

    nc.compile()
    return nc


def prepare(inputs):
    """Host-side prep: rank sections, route combos, build device arrays."""
    s = np.asarray(inputs["output"], np.float32)[:, :, 0]
    a1 = np.asarray(inputs["a1"], np.int64)
    a2 = np.asarray(inputs["a2"], np.int64)
    direct = np.asarray(inputs["direct"], np.int64)
    pad = np.asarray(inputs["pad_idx"], np.int64)
    seg = np.asarray(inputs["seg_len"], np.int64)
    NCv = direct.shape[0]
    lane = np.arange(L)[None, :]
    act = lane < seg[:, None]

    padrefs = np.bincount(pad[act], minlength=S)
    dirrefs = np.bincount(direct, minlength=S)
    usage = padrefs + dirrefs
    order = np.argsort(-usage, kind="stable")
    rank = np.empty(S, np.int64)
    rank[order] = np.arange(S)
    # partition row of each rank: top-32 -> 96..127 (dense strip),
    # next 96 -> 0..95 (scatter), rest -> 128.. (hi strip)
    pr_of_rank = np.concatenate([96 + np.arange(NBM), np.arange(NSC),
                                 128 + np.arange(S - 128)])
    prow = pr_of_rank[rank]                     # section -> partition row

    # collision rows (duplicate scatter targets) -> host
    a_ = np.where(act, pad, 2000 + lane)
    tcat = np.concatenate([np.where(direct < S, direct, 3000)[:, None], a_], 1)
    tcat.sort(axis=1)
    host = (tcat[:, 1:] == tcat[:, :-1]).any(1)

    hi_sec = np.zeros(S, bool)
    hi_sec[order[128:]] = True
    has_hi = hi_sec[direct] | (hi_sec[pad] & act).any(1)
    hi_mask = has_hi & ~host
    hi_idx = np.flatnonzero(hi_mask)
    if hi_idx.size > HI_CAP:
        host[hi_idx[HI_CAP:]] = True
        hi_idx = hi_idx[:HI_CAP]
    lo_idx = np.flatnonzero(~has_hi & ~host)

    rng = np.random.default_rng(9)
    core_of = np.full(NCv, -1, np.int64)
    tile_of = np.full(NCv, -1, np.int64)
    col_of = np.full(NCv, -1, np.int64)

    h = rng.permutation(hi_idx)
    hc = np.arange(h.size)
    core_of[h] = hc % CORES
    ht = hc // CORES
    if h.size:
        tile_of[h] = np.asarray(HI_TILES)[ht % N_HI_TILES]
        col_of[h] = ht // N_HI_TILES

    p = rng.permutation(lo_idx)
    used = np.zeros((CORES, T), np.int64)
    if h.size:
        np.add.at(used, (core_of[h], tile_of[h]), 1)
    free_cols = TILE - used
    cc, tt = np.meshgrid(np.arange(CORES), np.arange(T), indexing="ij")
    reps = free_cols.ravel()
    slot_core = np.repeat(cc.ravel(), reps)
    slot_tile = np.repeat(tt.ravel(), reps)
    slot_col = (np.arange(reps.sum()) -
                np.repeat(np.cumsum(reps) - reps, reps) +
                np.repeat(used.ravel(), reps))
    assert p.size <= slot_core.size, "combo overflow"
    n = p.size
    core_of[p] = slot_core[:n]
    tile_of[p] = slot_tile[:n]
    col_of[p] = slot_col[:n]

    is_dense_tile = np.asarray(_dense_flags)

    def build_entries():
        dev = np.flatnonzero(core_of >= 0)
        pr, pl = np.nonzero(act[dev])
        e_combo = np.concatenate([dev[pr], dev])
        e_row = np.concatenate([prow[pad[dev[pr], pl]], prow[direct[dev]]])
        e_val = np.concatenate([np.full(pr.size, -1.0, np.float32),
                                np.full(dev.size, 1.0, np.float32)])
        return (e_combo, e_row, e_val, core_of[e_combo], tile_of[e_combo],
                col_of[e_combo])

    e_combo, e_row, e_val, e_core, e_tile, e_col = build_entries()

    gp_pos_arr = np.full(T, -1, np.int64)
    for i, t_ in enumerate(GP_TILES):
        gp_pos_arr[t_] = i
    dn_pos_arr = np.full(T, -1, np.int64)
    for i, t_ in enumerate(DENSE_TILES):
        dn_pos_arr[t_] = i
    hi_pos_arr = np.full(T, -1, np.int64)
    for i, t_ in enumerate(HI_TILES):
        hi_pos_arr[t_] = i

    def prune(sel, cap, keyfn):
        nonlocal e_combo, e_row, e_val, e_core, e_tile, e_col, host
        idxs = np.flatnonzero(sel)
        if idxs.size == 0:
            return False
        key = keyfn(idxs)
        sort = np.argsort(key, kind="stable")
        ks = key[sort]
        first = np.r_[0, np.flatnonzero(np.diff(ks)) + 1]
        counts = np.diff(np.r_[first, ks.size])
        pos = np.arange(ks.size) - np.repeat(first, counts)
        bad = sort[pos >= cap]
        if bad.size == 0:
            return False
        bad_combos = np.unique(e_combo[idxs[bad]])
        host[bad_combos] = True
        core_of[bad_combos] = -1
        keep = core_of[e_combo] >= 0
        e_combo, e_row, e_val = e_combo[keep], e_row[keep], e_val[keep]
        e_core, e_tile, e_col = e_core[keep], e_tile[keep], e_col[keep]
        return True

    for _ in range(3):
        c1 = prune((e_row < NSC) & ~is_dense_tile[e_tile], NI,
                   lambda ii: (e_core[ii] * TG + gp_pos_arr[e_tile[ii]]) * NSC
                   + e_row[ii])
        c2 = prune(e_row >= 128, NIH,
                   lambda ii: (e_core[ii] * N_HI_TILES +
                               hi_pos_arr[e_tile[ii]]) * 32 +
                   (e_row[ii] - 128))
        if not (c1 or c2):
            break

    dense_e = is_dense_tile[e_tile]
    WD = np.zeros((CORES, TD, 128, TILE), ml_dtypes.bfloat16)
    de = np.flatnonzero(dense_e)
    WD[e_core[de], dn_pos_arr[e_tile[de]], e_row[de], e_col[de]] = \
        e_val[de].astype(ml_dtypes.bfloat16)
    WD = WD.view(np.int16)

    # dense strip rows (96..127) of gpsimd tiles, signs baked
    bm_e = (~dense_e) & (e_row >= NSC) & (e_row < 128)
    BMS = np.zeros((CORES, TG, NBM, TILE), ml_dtypes.bfloat16)
    be = np.flatnonzero(bm_e)
    BMS[e_core[be], gp_pos_arr[e_tile[be]], e_row[be] - NSC, e_col[be]] = \
        e_val[be].astype(ml_dtypes.bfloat16)
    BMS = BMS.view(np.int16)

    # scatter rows: combined idx|val int16 [CORES, TG, 96, 2*NI]
    sc = (~dense_e) & (e_row < NSC)
    se = np.flatnonzero(sc)
    skey = (e_core[se] * TG + gp_pos_arr[e_tile[se]]) * NSC + e_row[se]
    sort = np.argsort(skey, kind="stable")
    ks = skey[sort]
    _, first_idx, counts = np.unique(ks, return_index=True, return_counts=True)
    pos = np.arange(ks.size) - np.repeat(first_idx, counts)
    IXD = np.full((CORES, TG, NSC, 2 * NI), -1, np.int16)
    IXD[:, :, :, NI:] = 0
    ses = se[sort]
    IXD.reshape(-1, 2 * NI)[ks, pos] = e_col[ses].astype(np.int16)
    IXD.reshape(-1, 2 * NI)[ks, NI + pos] = \
        e_val[ses].astype(ml_dtypes.bfloat16).view(np.int16)

    # hi strip: combined idx|val [CORES, N_HI_TILES, 32, 2*NIH]
    HXD = np.full((CORES, N_HI_TILES, 32, 2 * NIH), -1, np.int16)
    HXD[:, :, :, NIH:] = 0
    he = np.flatnonzero(e_row >= 128)
    if he.size:
        hkey = ((e_core[he] * N_HI_TILES + hi_pos_arr[e_tile[he]]) * 32 +
                (e_row[he] - 128))
        sort = np.argsort(hkey, kind="stable")
        ks = hkey[sort]
        _, first_idx, counts = np.unique(ks, return_index=True,
                                         return_counts=True)
        pos = np.arange(ks.size) - np.repeat(first_idx, counts)
        hes = he[sort]
        HXD.reshape(-1, 2 * NIH)[ks, pos] = e_col[hes].astype(np.int16)
        HXD.reshape(-1, 2 * NIH)[ks, NIH + pos] = \
            e_val[hes].astype(ml_dtypes.bfloat16).view(np.int16)

    sTa = np.zeros((160, B), np.float32)
    sTa[prow] = s.T
    a12 = np.concatenate([prow[a1], prow[a2]]).astype(np.int32).reshape(
        1, 2 * NA)

    hs = np.flatnonzero(host)
    host_abs = 0.0
    if hs.size:
        m = act[hs].astype(np.float32)
        b2 = np.einsum("bnl,nl->bn", s[:, pad[hs]], m)
        b1 = s[:, direct[hs]]
        host_abs = float(np.abs(b1 - b2).sum())

    in_maps = []
    for c in range(CORES):
        m = {"sTa": sTa, "a12": a12, "WD": WD[c]}
        if TG:
            m.update({"BMS": BMS[c], "IXD": IXD[c]})
        if HI_TILES:
            m.update({"HXD": HXD[c]})
        in_maps.append(m)
    return in_maps, dict(NCv=NCv, host_abs=host_abs, n_host=int(hs.size))


def combine(outs, meta):
    total_abs = meta["host_abs"] + sum(float(outs[i]["outv"][:, 0].sum())
                                       for i in range(CORES))
    mean_a = float(np.exp(outs[0]["outv"][0:B, 1] / NA).mean())
    val = mean_a + total_abs / (B * meta["NCv"])
    return np.asarray(val, dtype=np.float32)


def get_nc():
    if "nc" not in _CACHE:
        _CACHE["nc"] = build_nc()
    return _CACHE["nc"]


def kernel(**inputs) -> np.ndarray:
    in_maps, meta = prepare(inputs)
    res = run_bass_kernel_spmd(get_nc(), in_maps, core_ids=list(range(CORES)))
    return combine(res.results, meta)
